# revision 5
# baseline (speedup 1.0000x reference)
"""3D Haar DWT (single level) on 8 Trainium2 NeuronCores.

Input:  data (2, 8, 128, 128, 128) f32 + six banded Haar matrices.
Output: tuple of 8 subbands (LLL, LLH, LHL, LHH, HLL, HLH, HHL, HHH),
        each (2, 8, 64, 64, 64) f32.  Band letters are [D][H][W] filters.

Strategy (per core, data-parallel over the 16 (n,c) slices -> 2 per core):
  - H-stage: PE matmul with stationary weights AH^T (low rows 0-63, high
    rows 64-127), pre-scaled by v_w*v_d so the W/D stages are pure
    unscaled butterflies.
  - D-stage: folded into PSUM accumulation: psum_lo = W@x[2e] + W@x[2e+1],
    psum_hi = W@x[2e] + (-W)@x[2e+1]  (second weight set is negated).
  - W-stage: DVE stride-2 add/sub pairs PSUM -> SBUF accumulation tiles.
  - Output: 4 accumulation tiles per slice ([p' 128][e 64][w' 64]), DMA'd
    contiguously; host splits p' halves into H-low/high bands.
"""

import sys

for _p in ("/opt/trn_rl_repo", "/root/.axon_site/_ro/trn_rl_repo"):
    if _p not in sys.path:
        sys.path.append(_p)

import json

import numpy as np

import concourse.bass as bass
import concourse.tile as tile
import concourse.mybir as mybir
from concourse.bass_utils import run_bass_kernel_spmd

N_CORES = 8
D = H = W = 128
SLICES_PER_CORE = 2           # (n,c) slices; N*C = 16 total
PLANES_PER_GROUP = 8          # depth planes per PE/DVE group (-> 4 output e's)
GROUPS_PER_SLICE = D // PLANES_PER_GROUP   # 16
F32 = mybir.dt.float32


# The pinned walrus build rejects instructions carrying more than one
# sync-wait ("Too many sync wait commands", CoreV3GenImpl setupSyncWait).
# Tile's wait assignment freely attaches several.  Post-process the
# serialized BIR: move all-but-one wait of any instruction onto fresh
# single-wait NoOps inserted just before it on the same engine (same
# per-engine program order -> identical semantics).
_orig_to_json_bytes = bass.Bass.to_json_bytes


def _split_multi_waits(data: bytes) -> bytes:
    d = json.loads(data)
    ctr = 0
    changed = False
    for f in d.get("functions", []):
        for blk in f.get("blocks", []):
            insts = blk.get("instructions", [])
            out = []
            for inst in insts:
                si = inst.get("sync_info") or {}
                ow = si.get("on_wait") or []
                if len(ow) > 1:
                    changed = True
                    for w in ow[:-1]:
                        ctr += 1
                        out.append(
                            {
                                "name": f"WS-{ctr}",
                                "opcode": "NoOp",
                                "engine": inst.get("engine"),
                                "ins": [],
                                "outs": [],
                                "debug": inst.get("debug"),
                                "sync_info": {
                                    "on_update": [],
                                    "on_wait": [w],
                                },
                            }
                        )
                    si["on_wait"] = [ow[-1]]
                out.append(inst)
            blk["instructions"] = out
    if not changed:
        return data
    return json.dumps(d).encode()


def _to_json_bytes_split(self):
    return _split_multi_waits(_orig_to_json_bytes(self))


bass.Bass.to_json_bytes = _to_json_bytes_split


def build_bass():
    """Build the per-core SPMD Bass program."""
    nc = bass.Bass("TRN2", target_bir_lowering=False, debug=False)

    x = nc.dram_tensor(
        "x", (SLICES_PER_CORE * D, H, W), F32, kind="ExternalInput"
    )
    wh = nc.dram_tensor("wh", (H, 128), F32, kind="ExternalInput")
    whn = nc.dram_tensor("whn", (H, 128), F32, kind="ExternalInput")
    y = nc.dram_tensor(
        "y", (SLICES_PER_CORE, 4, 128, D // 2 * (W // 2)), F32,
        kind="ExternalOutput",
    )

    with tile.TileContext(nc) as tc:
        with (
            tc.tile_pool(name="consts", bufs=1) as cpool,
            tc.tile_pool(name="inp", bufs=3) as ipool,
            tc.tile_pool(name="psum", bufs=4, space="PSUM") as ppool,
            tc.tile_pool(name="acc", bufs=2) as apool,
        ):
            wh_t = cpool.tile([H, 128], F32, tag="wh")
            nc.sync.dma_start(wh_t[:], wh.ap())
            whn_t = cpool.tile([H, 128], F32, tag="whn")
            nc.sync.dma_start(whn_t[:], whn.ap())

            for s in range(SLICES_PER_CORE):
                acc = [
                    apool.tile(
                        [128, D // 2 * (W // 2)], F32,
                        tag=f"acc{t}", name=f"acc{t}_{s}",
                    )
                    for t in range(4)
                ]
                for g in range(GROUPS_PER_SLICE):
                    p0 = s * D + g * PLANES_PER_GROUP
                    chunk = ipool.tile([H, PLANES_PER_GROUP * W], F32, tag="chunk")
                    nc.sync.dma_start(
                        chunk[:].rearrange("h (d w) -> h d w", w=W),
                        x.ap()[p0 : p0 + PLANES_PER_GROUP].rearrange(
                            "d h w -> h d w"
                        ),
                    )
                    planes = chunk[:].rearrange("h (d w) -> h d w", w=W)
                    even = planes[:, 0::2, :]   # [128, 4, 128] free=512
                    odd = planes[:, 1::2, :]

                    psum_lo = ppool.tile([128, 512], F32, tag="lo")
                    psum_hi = ppool.tile([128, 512], F32, tag="hi")
                    nc.tensor.matmul(psum_lo[:], wh_t[:], even, start=True, stop=False)
                    nc.tensor.matmul(psum_lo[:], wh_t[:], odd, start=False, stop=True)
                    nc.tensor.matmul(psum_hi[:], wh_t[:], even, start=True, stop=False)
                    nc.tensor.matmul(psum_hi[:], whn_t[:], odd, start=False, stop=True)

                    # TT can't read two PSUM operands; stage through SBUF
                    # on the otherwise-idle ScalarE.
                    sb_lo = ipool.tile([128, 512], F32, tag="sblo", name="sb_lo")
                    nc.scalar.copy(sb_lo[:], psum_lo[:])
                    sb_hi = ipool.tile([128, 512], F32, tag="sbhi", name="sb_hi")
                    nc.scalar.copy(sb_hi[:], psum_hi[:])

                    ecount = PLANES_PER_GROUP // 2  # output e's in this group
                    for src, t_sum, t_diff in ((sb_lo, 0, 1), (sb_hi, 2, 3)):
                        r = src[:].rearrange(
                            "p (e w two) -> p e w two", two=2, w=W // 2
                        )
                        ev = r[:, :, :, 0]
                        od = r[:, :, :, 1]
                        off = g * ecount * (W // 2)
                        for t_out, op in ((t_sum, "add"), (t_diff, "sub")):
                            out = acc[t_out][:, off : off + ecount * (W // 2)]
                            out = out.rearrange("p (e w) -> p e w", w=W // 2)
                            if op == "add":
                                nc.vector.tensor_add(out, ev, od)
                            else:
                                nc.vector.tensor_sub(out, ev, od)

                for t in range(4):
                    nc.sync.dma_start(y.ap()[s, t], acc[t][:])

    return nc


_NC_CACHE = None


def _get_nc():
    global _NC_CACHE
    if _NC_CACHE is None:
        _NC_CACHE = build_bass()
    return _NC_CACHE


def _host_prep(inputs):
    l0 = np.asarray(inputs["matrix_low_0"], dtype=np.float64)   # (64,128)
    g0 = np.asarray(inputs["matrix_high_0"], dtype=np.float64)  # (64,128)
    l1 = np.asarray(inputs["matrix_low_1"], dtype=np.float64)   # (128,64)
    l2 = np.asarray(inputs["matrix_low_2"], dtype=np.float64)   # (64,128)
    v_w = l1[0, 0]
    v_d = l2[0, 0]
    ah = np.concatenate([l0, g0], axis=0)          # (128, 128) rows=bands
    wh = np.ascontiguousarray((ah.T * (v_w * v_d)).astype(np.float32))
    whn = np.ascontiguousarray(-wh)
    return wh, whn


def run(inputs, trace=False, **kwargs):
    """Run the kernel; returns (bands_tuple, BassKernelResults)."""
    data = np.asarray(inputs["data"])
    assert data.shape == (2, 8, D, H, W) and data.dtype == np.float32
    wh, whn = _host_prep(inputs)

    x = np.ascontiguousarray(data.reshape(16, D, H, W))
    in_maps = []
    for k in range(N_CORES):
        xs = np.ascontiguousarray(
            x[2 * k : 2 * k + 2].reshape(SLICES_PER_CORE * D, H, W)
        )
        in_maps.append({"x": xs, "wh": wh, "whn": whn})

    nc = _get_nc()
    res = run_bass_kernel_spmd(
        nc, in_maps, core_ids=list(range(N_CORES)), trace=trace, **kwargs
    )

    # Reassemble bands: y[k] is (2, 4, 128, 4096) ->
    # [slice][tile t=2*d_hi + w_hi][p' (h band halves)][e*64 + w'].
    bands = [np.empty((2, 8, D // 2, H // 2, W // 2), np.float32) for _ in range(8)]
    for k in range(N_CORES):
        yk = res.results[k]["y"].reshape(SLICES_PER_CORE, 4, 128, D // 2, W // 2)
        for s in range(SLICES_PER_CORE):
            ncf = 2 * k + s
            n, c = divmod(ncf, 8)
            for d_hi in (0, 1):
                for w_hi in (0, 1):
                    t = 2 * d_hi + w_hi
                    for h_hi in (0, 1):
                        band = 4 * d_hi + 2 * h_hi + w_hi
                        blk = yk[s, t, 64 * h_hi : 64 * h_hi + 64]  # [p', e, w']
                        bands[band][n, c] = blk.transpose(1, 0, 2)
    return tuple(bands), res


def kernel(**inputs):
    out, _ = run(inputs)
    return out


# revision 8
# speedup vs baseline: 1.0149x; 1.0149x over previous
"""3D Haar DWT (single level) on 8 Trainium2 NeuronCores.

Input:  data (2, 8, 128, 128, 128) f32 + six banded Haar matrices.
Output: tuple of 8 subbands (LLL, LLH, LHL, LHH, HLL, HLH, HHL, HHH),
        each (2, 8, 64, 64, 64) f32.  Band letters are [D][H][W] filters.

Strategy (per core, data-parallel over the 16 (n,c) slices -> 2 per core):
  - H-stage: PE matmul with stationary weights AH^T (low rows 0-63, high
    rows 64-127), pre-scaled by v_w*v_d so the W/D stages are pure
    unscaled butterflies.
  - D-stage: folded into PSUM accumulation: psum_lo = W@x[2e] + W@x[2e+1],
    psum_hi = W@x[2e] + (-W)@x[2e+1]  (second weight set is negated).
  - W-stage: DVE stride-2 add/sub pairs PSUM -> SBUF accumulation tiles.
  - Output: 4 accumulation tiles per slice ([p' 128][e 64][w' 64]), DMA'd
    contiguously; host splits p' halves into H-low/high bands.
"""

import sys

for _p in ("/opt/trn_rl_repo", "/root/.axon_site/_ro/trn_rl_repo"):
    if _p not in sys.path:
        sys.path.append(_p)

import json

import numpy as np

import concourse.bass as bass
import concourse.tile as tile
import concourse.mybir as mybir
from concourse.bass_utils import run_bass_kernel_spmd

N_CORES = 8
D = H = W = 128
SLICES_PER_CORE = 2           # (n,c) slices; N*C = 16 total
PLANES_PER_GROUP = 8          # depth planes per PE/DVE group (-> 4 output e's)
GROUPS_PER_SLICE = D // PLANES_PER_GROUP   # 16
F32 = mybir.dt.float32


# The pinned walrus build rejects instructions carrying more than one
# sync-wait ("Too many sync wait commands", CoreV3GenImpl setupSyncWait).
# Tile's wait assignment freely attaches several.  Post-process the
# serialized BIR: move all-but-one wait of any instruction onto fresh
# single-wait NoOps inserted just before it on the same engine (same
# per-engine program order -> identical semantics).
_orig_to_json_bytes = bass.Bass.to_json_bytes


def _split_multi_waits(data: bytes) -> bytes:
    d = json.loads(data)
    ctr = 0
    changed = False
    for f in d.get("functions", []):
        for blk in f.get("blocks", []):
            insts = blk.get("instructions", [])
            out = []
            for inst in insts:
                si = inst.get("sync_info") or {}
                ow = si.get("on_wait") or []
                if len(ow) > 1:
                    changed = True
                    for w in ow[:-1]:
                        ctr += 1
                        out.append(
                            {
                                "name": f"WS-{ctr}",
                                "opcode": "NoOp",
                                "engine": inst.get("engine"),
                                "ins": [],
                                "outs": [],
                                "debug": inst.get("debug"),
                                "sync_info": {
                                    "on_update": [],
                                    "on_wait": [w],
                                },
                            }
                        )
                    si["on_wait"] = [ow[-1]]
                out.append(inst)
            blk["instructions"] = out
    if not changed:
        return data
    return json.dumps(d).encode()


def _to_json_bytes_split(self):
    return _split_multi_waits(_orig_to_json_bytes(self))


bass.Bass.to_json_bytes = _to_json_bytes_split


def build_bass():
    """Build the per-core SPMD Bass program."""
    nc = bass.Bass("TRN2", target_bir_lowering=False, debug=False)

    # x is host-pre-transposed to [slice][h][d][w] so input DMAs read
    # 4 KiB contiguous per partition (dense HBM bursts).
    x = nc.dram_tensor(
        "x", (SLICES_PER_CORE, H, D, W), F32, kind="ExternalInput"
    )
    wh = nc.dram_tensor("wh", (H, 128), F32, kind="ExternalInput")
    whn = nc.dram_tensor("whn", (H, 128), F32, kind="ExternalInput")
    y = nc.dram_tensor(
        "y", (SLICES_PER_CORE, 4, 128, D // 2 * (W // 2)), F32,
        kind="ExternalOutput",
    )

    with tile.TileContext(nc) as tc:
        with (
            tc.tile_pool(name="consts", bufs=1) as cpool,
            tc.tile_pool(name="inp", bufs=3) as ipool,
            tc.tile_pool(name="psum", bufs=4, space="PSUM") as ppool,
            tc.tile_pool(name="acc", bufs=2) as apool,
        ):
            wh_t = cpool.tile([H, 128], F32, tag="wh")
            nc.sync.dma_start(wh_t[:], wh.ap())
            whn_t = cpool.tile([H, 128], F32, tag="whn")
            nc.sync.dma_start(whn_t[:], whn.ap())

            CH = 2 * PLANES_PER_GROUP  # 16 planes (1 MiB) per input DMA
            for s in range(SLICES_PER_CORE):
                acc = [
                    apool.tile(
                        [128, D // 2 * (W // 2)], F32,
                        tag=f"acc{t}", name=f"acc{t}_{s}",
                    )
                    for t in range(4)
                ]
                for c in range(D // CH):
                    chunk = ipool.tile([H, CH * W], F32, tag="chunk", name="chunk")
                    nc.sync.dma_start(
                        chunk[:].rearrange("h (d w) -> h d w", w=W),
                        x.ap()[s][:, c * CH : (c + 1) * CH, :],
                    )
                    planes_all = chunk[:].rearrange("h (d w) -> h d w", w=W)
                    for g2 in range(CH // PLANES_PER_GROUP):
                        g = c * (CH // PLANES_PER_GROUP) + g2
                        planes = planes_all[
                            :, g2 * PLANES_PER_GROUP : (g2 + 1) * PLANES_PER_GROUP, :
                        ]
                        even = planes[:, 0::2, :]   # [128, 4, 128] free=512
                        odd = planes[:, 1::2, :]

                        psum_lo = ppool.tile([128, 512], F32, tag="lo", name="pl")
                        psum_hi = ppool.tile([128, 512], F32, tag="hi", name="ph")
                        nc.tensor.matmul(psum_lo[:], wh_t[:], even, start=True, stop=False)
                        nc.tensor.matmul(psum_lo[:], wh_t[:], odd, start=False, stop=True)
                        nc.tensor.matmul(psum_hi[:], wh_t[:], even, start=True, stop=False)
                        nc.tensor.matmul(psum_hi[:], whn_t[:], odd, start=False, stop=True)

                        # W-stage butterfly: TT may read only one PSUM
                        # operand, so ScalarE stages the odd elements to
                        # SBUF; DVE reads the even elements from PSUM.
                        ecount = PLANES_PER_GROUP // 2  # e's per group
                        off = g * ecount * (W // 2)
                        for src, t_sum, t_diff, nm in (
                            (psum_lo, 0, 1, "lo"),
                            (psum_hi, 2, 3, "hi"),
                        ):
                            r = src[:].rearrange(
                                "p (e w two) -> p e w two", two=2, w=W // 2
                            )
                            ev = r[:, :, :, 0]
                            od = r[:, :, :, 1]
                            sb_od = ipool.tile(
                                [128, ecount * (W // 2)], F32,
                                tag=f"sbod_{nm}", name=f"sb_od_{nm}",
                            )
                            od_sb = sb_od[:].rearrange(
                                "p (e w) -> p e w", w=W // 2
                            )
                            nc.scalar.copy(od_sb, od)
                            out_sum = acc[t_sum][
                                :, off : off + ecount * (W // 2)
                            ].rearrange("p (e w) -> p e w", w=W // 2)
                            out_diff = acc[t_diff][
                                :, off : off + ecount * (W // 2)
                            ].rearrange("p (e w) -> p e w", w=W // 2)
                            nc.vector.tensor_add(out_sum, ev, od_sb)
                            nc.vector.tensor_sub(out_diff, ev, od_sb)

                for t in range(4):
                    nc.sync.dma_start(y.ap()[s, t], acc[t][:])

    return nc


_NC_CACHE = None


def _get_nc():
    global _NC_CACHE
    if _NC_CACHE is None:
        _NC_CACHE = build_bass()
    return _NC_CACHE


def _host_prep(inputs):
    l0 = np.asarray(inputs["matrix_low_0"], dtype=np.float64)   # (64,128)
    g0 = np.asarray(inputs["matrix_high_0"], dtype=np.float64)  # (64,128)
    l1 = np.asarray(inputs["matrix_low_1"], dtype=np.float64)   # (128,64)
    l2 = np.asarray(inputs["matrix_low_2"], dtype=np.float64)   # (64,128)
    v_w = l1[0, 0]
    v_d = l2[0, 0]
    ah = np.concatenate([l0, g0], axis=0)          # (128, 128) rows=bands
    wh = np.ascontiguousarray((ah.T * (v_w * v_d)).astype(np.float32))
    whn = np.ascontiguousarray(-wh)
    return wh, whn


def run(inputs, trace=False, **kwargs):
    """Run the kernel; returns (bands_tuple, BassKernelResults)."""
    data = np.asarray(inputs["data"])
    assert data.shape == (2, 8, D, H, W) and data.dtype == np.float32
    wh, whn = _host_prep(inputs)

    x = data.reshape(16, D, H, W)
    in_maps = []
    for k in range(N_CORES):
        # [s][d][h][w] -> [s][h][d][w] so device DMAs are dense
        xs = np.ascontiguousarray(x[2 * k : 2 * k + 2].transpose(0, 2, 1, 3))
        in_maps.append({"x": xs, "wh": wh, "whn": whn})

    nc = _get_nc()
    res = run_bass_kernel_spmd(
        nc, in_maps, core_ids=list(range(N_CORES)), trace=trace, **kwargs
    )

    # Reassemble bands: y[k] is (2, 4, 128, 4096) ->
    # [slice][tile t=2*d_hi + w_hi][p' (h band halves)][e*64 + w'].
    bands = [np.empty((2, 8, D // 2, H // 2, W // 2), np.float32) for _ in range(8)]
    for k in range(N_CORES):
        yk = res.results[k]["y"].reshape(SLICES_PER_CORE, 4, 128, D // 2, W // 2)
        for s in range(SLICES_PER_CORE):
            ncf = 2 * k + s
            n, c = divmod(ncf, 8)
            for d_hi in (0, 1):
                for w_hi in (0, 1):
                    t = 2 * d_hi + w_hi
                    for h_hi in (0, 1):
                        band = 4 * d_hi + 2 * h_hi + w_hi
                        blk = yk[s, t, 64 * h_hi : 64 * h_hi + 64]  # [p', e, w']
                        bands[band][n, c] = blk.transpose(1, 0, 2)
    return tuple(bands), res


def kernel(**inputs):
    out, _ = run(inputs)
    return out


# revision 13
# speedup vs baseline: 1.3135x; 1.2942x over previous
"""3D Haar DWT (single level) on 8 Trainium2 NeuronCores.

Input:  data (2, 8, 128, 128, 128) f32 + six banded Haar matrices.
Output: tuple of 8 subbands (LLL, LLH, LHL, LHH, HLL, HLH, HHL, HHH),
        each (2, 8, 64, 64, 64) f32.  Band letters are [D][H][W] filters.

Strategy (per core, data-parallel over the 16 (n,c) slices -> 2 per core):
  - H-stage: PE matmul with stationary weights AH^T (low rows 0-63, high
    rows 64-127), pre-scaled by v_w*v_d so the W/D stages are pure
    unscaled butterflies.
  - D-stage: folded into PSUM accumulation: psum_lo = W@x[2e] + W@x[2e+1],
    psum_hi = W@x[2e] + (-W)@x[2e+1]  (second weight set is negated).
  - W-stage: DVE stride-2 add/sub pairs PSUM -> SBUF accumulation tiles.
  - Output: 4 accumulation tiles per slice ([p' 128][e 64][w' 64]), DMA'd
    contiguously; host splits p' halves into H-low/high bands.
"""

import sys

for _p in ("/opt/trn_rl_repo", "/root/.axon_site/_ro/trn_rl_repo"):
    if _p not in sys.path:
        sys.path.append(_p)

import json

import numpy as np

import concourse.bass as bass
import concourse.tile as tile
import concourse.mybir as mybir
from concourse.bass_utils import run_bass_kernel_spmd

N_CORES = 8
D = H = W = 128
SLICES_PER_CORE = 2           # (n,c) slices; N*C = 16 total
PLANES_PER_GROUP = 8          # depth planes per PE/DVE group (-> 4 output e's)
GROUPS_PER_SLICE = D // PLANES_PER_GROUP   # 16
F32 = mybir.dt.float32


# The pinned walrus build rejects instructions carrying more than one
# sync-wait ("Too many sync wait commands", CoreV3GenImpl setupSyncWait).
# Tile's wait assignment freely attaches several.  Post-process the
# serialized BIR: move all-but-one wait of any instruction onto fresh
# single-wait NoOps inserted just before it on the same engine (same
# per-engine program order -> identical semantics).
_orig_to_json_bytes = bass.Bass.to_json_bytes


def _split_multi_waits(data: bytes) -> bytes:
    d = json.loads(data)
    ctr = 0
    changed = False
    for f in d.get("functions", []):
        for blk in f.get("blocks", []):
            insts = blk.get("instructions", [])
            out = []
            for inst in insts:
                si = inst.get("sync_info") or {}
                ow = si.get("on_wait") or []
                if len(ow) > 1:
                    changed = True
                    for w in ow[:-1]:
                        ctr += 1
                        out.append(
                            {
                                "name": f"WS-{ctr}",
                                "opcode": "NoOp",
                                "engine": inst.get("engine"),
                                "ins": [],
                                "outs": [],
                                "debug": inst.get("debug"),
                                "sync_info": {
                                    "on_update": [],
                                    "on_wait": [w],
                                },
                            }
                        )
                    si["on_wait"] = [ow[-1]]
                out.append(inst)
            blk["instructions"] = out
    if not changed:
        return data
    return json.dumps(d).encode()


def _to_json_bytes_split(self):
    return _split_multi_waits(_orig_to_json_bytes(self))


bass.Bass.to_json_bytes = _to_json_bytes_split


def build_bass():
    """Build the per-core SPMD Bass program."""
    nc = bass.Bass("TRN2", target_bir_lowering=False, debug=False)

    # x is host-pre-transposed to [slice][h][d][w] so input DMAs read
    # 4 KiB contiguous per partition (dense HBM bursts).
    x = nc.dram_tensor(
        "x", (SLICES_PER_CORE, H, D, W), F32, kind="ExternalInput"
    )
    wh = nc.dram_tensor("wh", (H, 128), F32, kind="ExternalInput")
    y = nc.dram_tensor(
        "y", (SLICES_PER_CORE, 4, 128, D // 2 * (W // 2)), F32,
        kind="ExternalOutput",
    )

    with tile.TileContext(nc) as tc:
        with (
            tc.tile_pool(name="consts", bufs=1) as cpool,
            tc.tile_pool(name="inp", bufs=3) as ipool,
            tc.tile_pool(name="psum", bufs=2, space="PSUM") as ppool,
            tc.tile_pool(name="acc", bufs=2) as apool,
        ):
            wh_t = cpool.tile([H, 128], F32, tag="wh")
            nc.sync.dma_start(wh_t[:], wh.ap())

            CH = 16                   # planes per chunk (1 MiB input DMA)
            E_CH = CH // 2            # 8 output e's per chunk
            QC = 4                    # chunks per output flush
            n_chunks = D // CH        # 8 per slice
            for s in range(SLICES_PER_CORE):
                for q in range(n_chunks // QC):
                    acc = [
                        apool.tile(
                            [128, QC * E_CH * (W // 2)], F32,
                            tag=f"acc{t}", name=f"acc{t}_{s}_{q}",
                        )
                        for t in range(4)
                    ]
                    for c2 in range(QC):
                        c = q * QC + c2
                        chunk = ipool.tile(
                            [H, CH * W], F32, tag="chunk", name="chunk"
                        )
                        nc.sync.dma_start(
                            chunk[:].rearrange("h (d w) -> h d w", w=W),
                            x.ap()[s][:, c * CH : (c + 1) * CH, :],
                        )
                        planes = chunk[:].rearrange("h (d w) -> h d w", w=W)
                        d_even = planes[:, 0::2, :]   # [128, 8, 128]
                        d_odd = planes[:, 1::2, :]

                        # depth butterfly on raw input (SBUF->SBUF):
                        # sums on DVE, diffs on GpSimd
                        dsum = ipool.tile([H, E_CH * W], F32, tag="dsum", name="dsum")
                        ddiff = ipool.tile([H, E_CH * W], F32, tag="ddiff", name="ddiff")
                        nc.vector.tensor_add(
                            dsum[:].rearrange("h (e w) -> h e w", w=W),
                            d_even, d_odd,
                        )
                        nc.gpsimd.tensor_sub(
                            ddiff[:].rearrange("h (e w) -> h e w", w=W),
                            d_even, d_odd,
                        )

                        # H-stage matmuls (single weight set; fp32 N<=512)
                        psum_lo = ppool.tile([128, 1024], F32, tag="lo", name="pl")
                        psum_hi = ppool.tile([128, 1024], F32, tag="hi", name="ph")
                        for half in range(2):
                            sl = slice(half * 512, (half + 1) * 512)
                            nc.tensor.matmul(
                                psum_lo[:, sl], wh_t[:], dsum[:, sl],
                                start=True, stop=True,
                            )
                            nc.tensor.matmul(
                                psum_hi[:, sl], wh_t[:], ddiff[:, sl],
                                start=True, stop=True,
                            )

                        # W-stage butterfly: TT may read only one PSUM
                        # operand, so ScalarE stages the odd elements to
                        # SBUF; DVE reads the even elements from PSUM.
                        off = c2 * E_CH * (W // 2)
                        for src, t_sum, t_diff, nm in (
                            (psum_lo, 0, 1, "lo"),
                            (psum_hi, 2, 3, "hi"),
                        ):
                            r = src[:].rearrange(
                                "p (e w two) -> p e w two", two=2, w=W // 2
                            )
                            ev = r[:, :, :, 0]
                            od = r[:, :, :, 1]
                            sb_od = ipool.tile(
                                [128, E_CH * (W // 2)], F32,
                                tag=f"sbod_{nm}", name=f"sb_od_{nm}",
                            )
                            od_sb = sb_od[:].rearrange(
                                "p (e w) -> p e w", w=W // 2
                            )
                            nc.scalar.copy(od_sb, od)
                            out_sum = acc[t_sum][
                                :, off : off + E_CH * (W // 2)
                            ].rearrange("p (e w) -> p e w", w=W // 2)
                            out_diff = acc[t_diff][
                                :, off : off + E_CH * (W // 2)
                            ].rearrange("p (e w) -> p e w", w=W // 2)
                            nc.vector.tensor_add(out_sum, ev, od_sb)
                            nc.vector.tensor_sub(out_diff, ev, od_sb)

                    flush = QC * E_CH * (W // 2)   # 2048 elems/partition
                    for t in range(4):
                        nc.sync.dma_start(
                            y.ap()[s, t][:, q * flush : (q + 1) * flush],
                            acc[t][:],
                        )

    return nc


_NC_CACHE = None


def _get_nc():
    global _NC_CACHE
    if _NC_CACHE is None:
        _NC_CACHE = build_bass()
    return _NC_CACHE


def _host_prep(inputs):
    l0 = np.asarray(inputs["matrix_low_0"], dtype=np.float64)   # (64,128)
    g0 = np.asarray(inputs["matrix_high_0"], dtype=np.float64)  # (64,128)
    l1 = np.asarray(inputs["matrix_low_1"], dtype=np.float64)   # (128,64)
    l2 = np.asarray(inputs["matrix_low_2"], dtype=np.float64)   # (64,128)
    v_w = l1[0, 0]
    v_d = l2[0, 0]
    ah = np.concatenate([l0, g0], axis=0)          # (128, 128) rows=bands
    wh = np.ascontiguousarray((ah.T * (v_w * v_d)).astype(np.float32))
    return wh


def run(inputs, trace=False, **kwargs):
    """Run the kernel; returns (bands_tuple, BassKernelResults)."""
    data = np.asarray(inputs["data"])
    assert data.shape == (2, 8, D, H, W) and data.dtype == np.float32
    wh = _host_prep(inputs)

    x = data.reshape(16, D, H, W)
    in_maps = []
    for k in range(N_CORES):
        # [s][d][h][w] -> [s][h][d][w] so device DMAs are dense
        xs = np.ascontiguousarray(x[2 * k : 2 * k + 2].transpose(0, 2, 1, 3))
        in_maps.append({"x": xs, "wh": wh})

    nc = _get_nc()
    res = run_bass_kernel_spmd(
        nc, in_maps, core_ids=list(range(N_CORES)), trace=trace, **kwargs
    )

    # Reassemble bands: y[k] is (2, 4, 128, 4096) ->
    # [slice][tile t=2*d_hi + w_hi][p' (h band halves)][e*64 + w'].
    bands = [np.empty((2, 8, D // 2, H // 2, W // 2), np.float32) for _ in range(8)]
    for k in range(N_CORES):
        yk = res.results[k]["y"].reshape(SLICES_PER_CORE, 4, 128, D // 2, W // 2)
        for s in range(SLICES_PER_CORE):
            ncf = 2 * k + s
            n, c = divmod(ncf, 8)
            for d_hi in (0, 1):
                for w_hi in (0, 1):
                    t = 2 * d_hi + w_hi
                    for h_hi in (0, 1):
                        band = 4 * d_hi + 2 * h_hi + w_hi
                        blk = yk[s, t, 64 * h_hi : 64 * h_hi + 64]  # [p', e, w']
                        bands[band][n, c] = blk.transpose(1, 0, 2)
    return tuple(bands), res


def kernel(**inputs):
    out, _ = run(inputs)
    return out


# revision 14
# speedup vs baseline: 1.3425x; 1.0221x over previous
"""3D Haar DWT (single level) on 8 Trainium2 NeuronCores.

Input:  data (2, 8, 128, 128, 128) f32 + six banded Haar matrices.
Output: tuple of 8 subbands (LLL, LLH, LHL, LHH, HLL, HLH, HHL, HHH),
        each (2, 8, 64, 64, 64) f32.  Band letters are [D][H][W] filters.

Strategy (per core, data-parallel over the 16 (n,c) slices -> 2 per core):
  - H-stage: PE matmul with stationary weights AH^T (low rows 0-63, high
    rows 64-127), pre-scaled by v_w*v_d so the W/D stages are pure
    unscaled butterflies.
  - D-stage: folded into PSUM accumulation: psum_lo = W@x[2e] + W@x[2e+1],
    psum_hi = W@x[2e] + (-W)@x[2e+1]  (second weight set is negated).
  - W-stage: DVE stride-2 add/sub pairs PSUM -> SBUF accumulation tiles.
  - Output: 4 accumulation tiles per slice ([p' 128][e 64][w' 64]), DMA'd
    contiguously; host splits p' halves into H-low/high bands.
"""

import sys

for _p in ("/opt/trn_rl_repo", "/root/.axon_site/_ro/trn_rl_repo"):
    if _p not in sys.path:
        sys.path.append(_p)

import json

import numpy as np

import concourse.bass as bass
import concourse.tile as tile
import concourse.mybir as mybir
from concourse.bass_utils import run_bass_kernel_spmd

N_CORES = 8
D = H = W = 128
SLICES_PER_CORE = 2           # (n,c) slices; N*C = 16 total
PLANES_PER_GROUP = 8          # depth planes per PE/DVE group (-> 4 output e's)
GROUPS_PER_SLICE = D // PLANES_PER_GROUP   # 16
F32 = mybir.dt.float32


# The pinned walrus build rejects instructions carrying more than one
# sync-wait ("Too many sync wait commands", CoreV3GenImpl setupSyncWait).
# Tile's wait assignment freely attaches several.  Post-process the
# serialized BIR: move all-but-one wait of any instruction onto fresh
# single-wait NoOps inserted just before it on the same engine (same
# per-engine program order -> identical semantics).
_orig_to_json_bytes = bass.Bass.to_json_bytes


def _split_multi_waits(data: bytes) -> bytes:
    d = json.loads(data)
    ctr = 0
    changed = False
    for f in d.get("functions", []):
        for blk in f.get("blocks", []):
            insts = blk.get("instructions", [])
            out = []
            for inst in insts:
                si = inst.get("sync_info") or {}
                ow = si.get("on_wait") or []
                if len(ow) > 1:
                    changed = True
                    for w in ow[:-1]:
                        ctr += 1
                        out.append(
                            {
                                "name": f"WS-{ctr}",
                                "opcode": "NoOp",
                                "engine": inst.get("engine"),
                                "ins": [],
                                "outs": [],
                                "debug": inst.get("debug"),
                                "sync_info": {
                                    "on_update": [],
                                    "on_wait": [w],
                                },
                            }
                        )
                    si["on_wait"] = [ow[-1]]
                out.append(inst)
            blk["instructions"] = out
    if not changed:
        return data
    return json.dumps(d).encode()


def _to_json_bytes_split(self):
    return _split_multi_waits(_orig_to_json_bytes(self))


bass.Bass.to_json_bytes = _to_json_bytes_split


def build_bass():
    """Build the per-core SPMD Bass program."""
    nc = bass.Bass("TRN2", target_bir_lowering=False, debug=False)

    # x is host-pre-transposed to [slice][h][d][w] so input DMAs read
    # 4 KiB contiguous per partition (dense HBM bursts).
    x = nc.dram_tensor(
        "x", (SLICES_PER_CORE, H, D, W), F32, kind="ExternalInput"
    )
    wh = nc.dram_tensor("wh", (H, 128), F32, kind="ExternalInput")
    y = nc.dram_tensor(
        "y", (SLICES_PER_CORE, 4, 128, D // 2 * (W // 2)), F32,
        kind="ExternalOutput",
    )

    # chunk schedule per slice: (start_plane, n_planes). Slice 0 starts
    # with two 8-plane chunks so the compute pipeline fills early.
    sched0 = [(0, 8), (8, 8)] + [(16 * k, 16) for k in range(1, 8)]
    sched1 = [(16 * k, 16) for k in range(8)]
    PREFETCH = 3

    with tile.TileContext(nc) as tc:
        with (
            tc.tile_pool(name="consts", bufs=1) as cpool,
            tc.tile_pool(name="inp", bufs=4) as ipool,
            tc.tile_pool(name="psum", bufs=2, space="PSUM") as ppool,
            tc.tile_pool(name="acc", bufs=3) as apool,
        ):
            wh_t = cpool.tile([H, 128], F32, tag="wh")
            nc.sync.dma_start(wh_t[:], wh.ap())

            jobs = [(0, c) for c in sched0] + [(1, c) for c in sched1]
            tiles = {}

            def issue_in(j):
                s, (p0, np_) = jobs[j]
                t = ipool.tile([H, 16 * W], F32, tag="chunk", name="chunk", bufs=4)
                nc.sync.dma_start(
                    t[:, : np_ * W].rearrange("h (d w) -> h d w", w=W),
                    x.ap()[s][:, p0 : p0 + np_, :],
                )
                tiles[j] = t

            for j in range(min(PREFETCH, len(jobs))):
                issue_in(j)

            for j, (s, (p0, np_)) in enumerate(jobs):
                if j + PREFETCH < len(jobs):
                    issue_in(j + PREFETCH)
                chunk = tiles.pop(j)
                E = np_ // 2              # output e's in this chunk
                planes = chunk[:, : np_ * W].rearrange("h (d w) -> h d w", w=W)
                d_even = planes[:, 0::2, :]   # [128, E, 128]
                d_odd = planes[:, 1::2, :]

                # depth butterfly on raw input (SBUF->SBUF):
                # sums on DVE, diffs on GpSimd
                dsum = ipool.tile([H, 8 * W], F32, tag="dsum", name="dsum", bufs=3)
                ddiff = ipool.tile([H, 8 * W], F32, tag="ddiff", name="ddiff", bufs=3)
                nc.vector.tensor_add(
                    dsum[:, : E * W].rearrange("h (e w) -> h e w", w=W),
                    d_even, d_odd,
                )
                nc.gpsimd.tensor_sub(
                    ddiff[:, : E * W].rearrange("h (e w) -> h e w", w=W),
                    d_even, d_odd,
                )

                # H-stage matmuls (single weight set; fp32 N<=512)
                psum_lo = ppool.tile([128, 1024], F32, tag="lo", name="pl")
                psum_hi = ppool.tile([128, 1024], F32, tag="hi", name="ph")
                for half in range(E * W // 512):
                    sl = slice(half * 512, (half + 1) * 512)
                    nc.tensor.matmul(
                        psum_lo[:, sl], wh_t[:], dsum[:, sl],
                        start=True, stop=True,
                    )
                    nc.tensor.matmul(
                        psum_hi[:, sl], wh_t[:], ddiff[:, sl],
                        start=True, stop=True,
                    )

                # W-stage butterfly: TT may read only one PSUM operand, so
                # ScalarE stages the odd elements to SBUF; DVE reads the
                # even elements straight from PSUM.
                e0 = p0 // 2
                for src, t_sum, t_diff, nm in (
                    (psum_lo, 0, 1, "lo"),
                    (psum_hi, 2, 3, "hi"),
                ):
                    r = src[:, : E * W].rearrange(
                        "p (e w two) -> p e w two", two=2, w=W // 2
                    )
                    ev = r[:, :, :, 0]
                    od = r[:, :, :, 1]
                    sb_od = ipool.tile(
                        [128, 8 * (W // 2)], F32,
                        tag=f"sbod_{nm}", name=f"sb_od_{nm}", bufs=3,
                    )
                    od_sb = sb_od[:, : E * (W // 2)].rearrange(
                        "p (e w) -> p e w", w=W // 2
                    )
                    nc.scalar.copy(od_sb, od)
                    for t_out, is_sum in ((t_sum, True), (t_diff, False)):
                        acc = apool.tile(
                            [128, 8 * (W // 2)], F32,
                            tag=f"acc{t_out}", name=f"acc{t_out}", bufs=3,
                        )
                        out = acc[:, : E * (W // 2)].rearrange(
                            "p (e w) -> p e w", w=W // 2
                        )
                        if is_sum:
                            nc.vector.tensor_add(out, ev, od_sb)
                        else:
                            nc.vector.tensor_sub(out, ev, od_sb)
                        nc.sync.dma_start(
                            y.ap()[s, t_out][
                                :, e0 * (W // 2) : (e0 + E) * (W // 2)
                            ],
                            acc[:, : E * (W // 2)],
                        )

    return nc


_NC_CACHE = None


def _get_nc():
    global _NC_CACHE
    if _NC_CACHE is None:
        _NC_CACHE = build_bass()
    return _NC_CACHE


def _host_prep(inputs):
    l0 = np.asarray(inputs["matrix_low_0"], dtype=np.float64)   # (64,128)
    g0 = np.asarray(inputs["matrix_high_0"], dtype=np.float64)  # (64,128)
    l1 = np.asarray(inputs["matrix_low_1"], dtype=np.float64)   # (128,64)
    l2 = np.asarray(inputs["matrix_low_2"], dtype=np.float64)   # (64,128)
    v_w = l1[0, 0]
    v_d = l2[0, 0]
    ah = np.concatenate([l0, g0], axis=0)          # (128, 128) rows=bands
    wh = np.ascontiguousarray((ah.T * (v_w * v_d)).astype(np.float32))
    return wh


def run(inputs, trace=False, **kwargs):
    """Run the kernel; returns (bands_tuple, BassKernelResults)."""
    data = np.asarray(inputs["data"])
    assert data.shape == (2, 8, D, H, W) and data.dtype == np.float32
    wh = _host_prep(inputs)

    x = data.reshape(16, D, H, W)
    in_maps = []
    for k in range(N_CORES):
        # [s][d][h][w] -> [s][h][d][w] so device DMAs are dense
        xs = np.ascontiguousarray(x[2 * k : 2 * k + 2].transpose(0, 2, 1, 3))
        in_maps.append({"x": xs, "wh": wh})

    nc = _get_nc()
    res = run_bass_kernel_spmd(
        nc, in_maps, core_ids=list(range(N_CORES)), trace=trace, **kwargs
    )

    # Reassemble bands: y[k] is (2, 4, 128, 4096) ->
    # [slice][tile t=2*d_hi + w_hi][p' (h band halves)][e*64 + w'].
    bands = [np.empty((2, 8, D // 2, H // 2, W // 2), np.float32) for _ in range(8)]
    for k in range(N_CORES):
        yk = res.results[k]["y"].reshape(SLICES_PER_CORE, 4, 128, D // 2, W // 2)
        for s in range(SLICES_PER_CORE):
            ncf = 2 * k + s
            n, c = divmod(ncf, 8)
            for d_hi in (0, 1):
                for w_hi in (0, 1):
                    t = 2 * d_hi + w_hi
                    for h_hi in (0, 1):
                        band = 4 * d_hi + 2 * h_hi + w_hi
                        blk = yk[s, t, 64 * h_hi : 64 * h_hi + 64]  # [p', e, w']
                        bands[band][n, c] = blk.transpose(1, 0, 2)
    return tuple(bands), res


def kernel(**inputs):
    out, _ = run(inputs)
    return out


# revision 16
# speedup vs baseline: 1.4519x; 1.0815x over previous
"""3D Haar DWT (single level) on 8 Trainium2 NeuronCores.

Input:  data (2, 8, 128, 128, 128) f32 + six banded Haar matrices.
Output: tuple of 8 subbands (LLL, LLH, LHL, LHH, HLL, HLH, HHL, HHH),
        each (2, 8, 64, 64, 64) f32.  Band letters are [D][H][W] filters.

Strategy (per core, data-parallel over the 16 (n,c) slices -> 2 per core):
  - H-stage: PE matmul with stationary weights AH^T (low rows 0-63, high
    rows 64-127), pre-scaled by v_w*v_d so the W/D stages are pure
    unscaled butterflies.
  - D-stage: folded into PSUM accumulation: psum_lo = W@x[2e] + W@x[2e+1],
    psum_hi = W@x[2e] + (-W)@x[2e+1]  (second weight set is negated).
  - W-stage: DVE stride-2 add/sub pairs PSUM -> SBUF accumulation tiles.
  - Output: 4 accumulation tiles per slice ([p' 128][e 64][w' 64]), DMA'd
    contiguously; host splits p' halves into H-low/high bands.
"""

import sys

for _p in ("/opt/trn_rl_repo", "/root/.axon_site/_ro/trn_rl_repo"):
    if _p not in sys.path:
        sys.path.append(_p)

import json

import numpy as np

import concourse.bass as bass
import concourse.tile as tile
import concourse.mybir as mybir
from concourse.bass_utils import run_bass_kernel_spmd

N_CORES = 8
D = H = W = 128
SLICES_PER_CORE = 2           # (n,c) slices; N*C = 16 total
PLANES_PER_GROUP = 8          # depth planes per PE/DVE group (-> 4 output e's)
GROUPS_PER_SLICE = D // PLANES_PER_GROUP   # 16
F32 = mybir.dt.float32


# The pinned walrus build rejects instructions carrying more than one
# sync-wait ("Too many sync wait commands", CoreV3GenImpl setupSyncWait).
# Tile's wait assignment freely attaches several.  Post-process the
# serialized BIR: move all-but-one wait of any instruction onto fresh
# single-wait NoOps inserted just before it on the same engine (same
# per-engine program order -> identical semantics).
_orig_to_json_bytes = bass.Bass.to_json_bytes


def _split_multi_waits(data: bytes) -> bytes:
    d = json.loads(data)
    ctr = 0
    changed = False
    for f in d.get("functions", []):
        for blk in f.get("blocks", []):
            insts = blk.get("instructions", [])
            out = []
            for inst in insts:
                si = inst.get("sync_info") or {}
                ow = si.get("on_wait") or []
                if len(ow) > 1:
                    changed = True
                    for w in ow[:-1]:
                        ctr += 1
                        out.append(
                            {
                                "name": f"WS-{ctr}",
                                "opcode": "NoOp",
                                "engine": inst.get("engine"),
                                "ins": [],
                                "outs": [],
                                "debug": inst.get("debug"),
                                "sync_info": {
                                    "on_update": [],
                                    "on_wait": [w],
                                },
                            }
                        )
                    si["on_wait"] = [ow[-1]]
                out.append(inst)
            blk["instructions"] = out
    if not changed:
        return data
    return json.dumps(d).encode()


def _to_json_bytes_split(self):
    return _split_multi_waits(_orig_to_json_bytes(self))


bass.Bass.to_json_bytes = _to_json_bytes_split


def build_bass():
    """Build the per-core SPMD Bass program."""
    nc = bass.Bass("TRN2", target_bir_lowering=False, debug=False)

    # x is host-pre-transposed to [slice][h][d][w] so input DMAs read
    # 4 KiB contiguous per partition (dense HBM bursts).
    x = nc.dram_tensor(
        "x", (SLICES_PER_CORE, H, D, W), F32, kind="ExternalInput"
    )
    wh = nc.dram_tensor("wh", (H, 128), F32, kind="ExternalInput")
    y = nc.dram_tensor(
        "y", (SLICES_PER_CORE, 4, 128, D // 2 * (W // 2)), F32,
        kind="ExternalOutput",
    )

    # chunk schedule per slice: (start_plane, n_planes). Slice 0 starts
    # with two 8-plane chunks so the compute pipeline fills early.
    sched0 = [(0, 8), (8, 8)] + [(16 * k, 16) for k in range(1, 8)]
    sched1 = [(16 * k, 16) for k in range(8)]
    PREFETCH = 3

    with tile.TileContext(nc) as tc:
        with (
            tc.tile_pool(name="consts", bufs=1) as cpool,
            tc.tile_pool(name="inp", bufs=4) as ipool,
            tc.tile_pool(name="psum", bufs=2, space="PSUM") as ppool,
            tc.tile_pool(name="acc", bufs=3) as apool,
        ):
            # inputs ride the ACT HWDGE ring; outputs ride the SP ring, so
            # the two streams never FIFO-serialize behind each other.
            wh_t = cpool.tile([H, 128], F32, tag="wh")
            nc.scalar.dma_start(wh_t[:], wh.ap())

            jobs = [(0, c) for c in sched0] + [(1, c) for c in sched1]
            tiles = {}

            def issue_in(j):
                s, (p0, np_) = jobs[j]
                t = ipool.tile([H, 16 * W], F32, tag="chunk", name="chunk", bufs=4)
                nc.scalar.dma_start(
                    t[:, : np_ * W].rearrange("h (d w) -> h d w", w=W),
                    x.ap()[s][:, p0 : p0 + np_, :],
                )
                tiles[j] = t

            for j in range(min(PREFETCH, len(jobs))):
                issue_in(j)

            for j, (s, (p0, np_)) in enumerate(jobs):
                if j + PREFETCH < len(jobs):
                    issue_in(j + PREFETCH)
                chunk = tiles.pop(j)
                E = np_ // 2              # output e's in this chunk
                planes = chunk[:, : np_ * W].rearrange("h (d w) -> h d w", w=W)
                d_even = planes[:, 0::2, :]   # [128, E, 128]
                d_odd = planes[:, 1::2, :]

                # depth butterfly on raw input (SBUF->SBUF):
                # sums on DVE, diffs on GpSimd
                dsum = ipool.tile([H, 8 * W], F32, tag="dsum", name="dsum", bufs=3)
                ddiff = ipool.tile([H, 8 * W], F32, tag="ddiff", name="ddiff", bufs=3)
                nc.vector.tensor_add(
                    dsum[:, : E * W].rearrange("h (e w) -> h e w", w=W),
                    d_even, d_odd,
                )
                nc.gpsimd.tensor_sub(
                    ddiff[:, : E * W].rearrange("h (e w) -> h e w", w=W),
                    d_even, d_odd,
                )

                # H-stage matmuls (single weight set; fp32 N<=512)
                psum_lo = ppool.tile([128, 1024], F32, tag="lo", name="pl")
                psum_hi = ppool.tile([128, 1024], F32, tag="hi", name="ph")
                for half in range(E * W // 512):
                    sl = slice(half * 512, (half + 1) * 512)
                    nc.tensor.matmul(
                        psum_lo[:, sl], wh_t[:], dsum[:, sl],
                        start=True, stop=True,
                    )
                    nc.tensor.matmul(
                        psum_hi[:, sl], wh_t[:], ddiff[:, sl],
                        start=True, stop=True,
                    )

                # W-stage butterfly: TT may read only one PSUM operand, so
                # ScalarE stages the odd elements to SBUF; DVE reads the
                # even elements straight from PSUM.
                e0 = p0 // 2
                for src, t_sum, t_diff, nm in (
                    (psum_lo, 0, 1, "lo"),
                    (psum_hi, 2, 3, "hi"),
                ):
                    r = src[:, : E * W].rearrange(
                        "p (e w two) -> p e w two", two=2, w=W // 2
                    )
                    ev = r[:, :, :, 0]
                    od = r[:, :, :, 1]
                    sb_od = ipool.tile(
                        [128, 8 * (W // 2)], F32,
                        tag=f"sbod_{nm}", name=f"sb_od_{nm}", bufs=3,
                    )
                    od_sb = sb_od[:, : E * (W // 2)].rearrange(
                        "p (e w) -> p e w", w=W // 2
                    )
                    nc.scalar.copy(od_sb, od)
                    for t_out, is_sum in ((t_sum, True), (t_diff, False)):
                        acc = apool.tile(
                            [128, 8 * (W // 2)], F32,
                            tag=f"acc{t_out}", name=f"acc{t_out}", bufs=3,
                        )
                        out = acc[:, : E * (W // 2)].rearrange(
                            "p (e w) -> p e w", w=W // 2
                        )
                        if is_sum:
                            nc.vector.tensor_add(out, ev, od_sb)
                        else:
                            nc.vector.tensor_sub(out, ev, od_sb)
                        nc.sync.dma_start(
                            y.ap()[s, t_out][
                                :, e0 * (W // 2) : (e0 + E) * (W // 2)
                            ],
                            acc[:, : E * (W // 2)],
                        )

    return nc


_NC_CACHE = None


def _get_nc():
    global _NC_CACHE
    if _NC_CACHE is None:
        _NC_CACHE = build_bass()
    return _NC_CACHE


def _host_prep(inputs):
    l0 = np.asarray(inputs["matrix_low_0"], dtype=np.float64)   # (64,128)
    g0 = np.asarray(inputs["matrix_high_0"], dtype=np.float64)  # (64,128)
    l1 = np.asarray(inputs["matrix_low_1"], dtype=np.float64)   # (128,64)
    l2 = np.asarray(inputs["matrix_low_2"], dtype=np.float64)   # (64,128)
    v_w = l1[0, 0]
    v_d = l2[0, 0]
    ah = np.concatenate([l0, g0], axis=0)          # (128, 128) rows=bands
    wh = np.ascontiguousarray((ah.T * (v_w * v_d)).astype(np.float32))
    return wh


def run(inputs, trace=False, **kwargs):
    """Run the kernel; returns (bands_tuple, BassKernelResults)."""
    data = np.asarray(inputs["data"])
    assert data.shape == (2, 8, D, H, W) and data.dtype == np.float32
    wh = _host_prep(inputs)

    x = data.reshape(16, D, H, W)
    in_maps = []
    for k in range(N_CORES):
        # [s][d][h][w] -> [s][h][d][w] so device DMAs are dense
        xs = np.ascontiguousarray(x[2 * k : 2 * k + 2].transpose(0, 2, 1, 3))
        in_maps.append({"x": xs, "wh": wh})

    nc = _get_nc()
    res = run_bass_kernel_spmd(
        nc, in_maps, core_ids=list(range(N_CORES)), trace=trace, **kwargs
    )

    # Reassemble bands: y[k] is (2, 4, 128, 4096) ->
    # [slice][tile t=2*d_hi + w_hi][p' (h band halves)][e*64 + w'].
    bands = [np.empty((2, 8, D // 2, H // 2, W // 2), np.float32) for _ in range(8)]
    for k in range(N_CORES):
        yk = res.results[k]["y"].reshape(SLICES_PER_CORE, 4, 128, D // 2, W // 2)
        for s in range(SLICES_PER_CORE):
            ncf = 2 * k + s
            n, c = divmod(ncf, 8)
            for d_hi in (0, 1):
                for w_hi in (0, 1):
                    t = 2 * d_hi + w_hi
                    for h_hi in (0, 1):
                        band = 4 * d_hi + 2 * h_hi + w_hi
                        blk = yk[s, t, 64 * h_hi : 64 * h_hi + 64]  # [p', e, w']
                        bands[band][n, c] = blk.transpose(1, 0, 2)
    return tuple(bands), res


def kernel(**inputs):
    out, _ = run(inputs)
    return out


# revision 18
# speedup vs baseline: 1.5019x; 1.0344x over previous
"""3D Haar DWT (single level) on 8 Trainium2 NeuronCores.

Input:  data (2, 8, 128, 128, 128) f32 + six banded Haar matrices.
Output: tuple of 8 subbands (LLL, LLH, LHL, LHH, HLL, HLH, HHL, HHH),
        each (2, 8, 64, 64, 64) f32.  Band letters are [D][H][W] filters.

Strategy (per core, data-parallel over the 16 (n,c) slices -> 2 per core):
  - H-stage: PE matmul with stationary weights AH^T (low rows 0-63, high
    rows 64-127), pre-scaled by v_w*v_d so the W/D stages are pure
    unscaled butterflies.
  - D-stage: folded into PSUM accumulation: psum_lo = W@x[2e] + W@x[2e+1],
    psum_hi = W@x[2e] + (-W)@x[2e+1]  (second weight set is negated).
  - W-stage: DVE stride-2 add/sub pairs PSUM -> SBUF accumulation tiles.
  - Output: 4 accumulation tiles per slice ([p' 128][e 64][w' 64]), DMA'd
    contiguously; host splits p' halves into H-low/high bands.
"""

import sys

for _p in ("/opt/trn_rl_repo", "/root/.axon_site/_ro/trn_rl_repo"):
    if _p not in sys.path:
        sys.path.append(_p)

import json

import numpy as np

import concourse.bass as bass
import concourse.tile as tile
import concourse.mybir as mybir
from concourse.bass_utils import run_bass_kernel_spmd

N_CORES = 8
D = H = W = 128
SLICES_PER_CORE = 2           # (n,c) slices; N*C = 16 total
PLANES_PER_GROUP = 8          # depth planes per PE/DVE group (-> 4 output e's)
GROUPS_PER_SLICE = D // PLANES_PER_GROUP   # 16
F32 = mybir.dt.float32


# The pinned walrus build rejects instructions carrying more than one
# sync-wait ("Too many sync wait commands", CoreV3GenImpl setupSyncWait).
# Tile's wait assignment freely attaches several.  Post-process the
# serialized BIR: move all-but-one wait of any instruction onto fresh
# single-wait NoOps inserted just before it on the same engine (same
# per-engine program order -> identical semantics).
_orig_to_json_bytes = bass.Bass.to_json_bytes


def _split_multi_waits(data: bytes) -> bytes:
    d = json.loads(data)
    ctr = 0
    changed = False
    for f in d.get("functions", []):
        for blk in f.get("blocks", []):
            insts = blk.get("instructions", [])
            out = []
            for inst in insts:
                si = inst.get("sync_info") or {}
                ow = si.get("on_wait") or []
                if len(ow) > 1:
                    changed = True
                    for w in ow[:-1]:
                        ctr += 1
                        out.append(
                            {
                                "name": f"WS-{ctr}",
                                "opcode": "NoOp",
                                "engine": inst.get("engine"),
                                "ins": [],
                                "outs": [],
                                "debug": inst.get("debug"),
                                "sync_info": {
                                    "on_update": [],
                                    "on_wait": [w],
                                },
                            }
                        )
                    si["on_wait"] = [ow[-1]]
                out.append(inst)
            blk["instructions"] = out
    if not changed:
        return data
    return json.dumps(d).encode()


def _to_json_bytes_split(self):
    return _split_multi_waits(_orig_to_json_bytes(self))


bass.Bass.to_json_bytes = _to_json_bytes_split


def build_bass():
    """Build the per-core SPMD Bass program."""
    nc = bass.Bass("TRN2", target_bir_lowering=False, debug=False)

    # x is host-pre-transposed to [slice][h][d][w] so input DMAs read
    # 4 KiB contiguous per partition (dense HBM bursts).
    x = nc.dram_tensor(
        "x", (SLICES_PER_CORE, H, D, W), F32, kind="ExternalInput"
    )
    wh = nc.dram_tensor("wh", (H, 128), F32, kind="ExternalInput")
    y = nc.dram_tensor(
        "y", (SLICES_PER_CORE, 4, 128, D // 2 * (W // 2)), F32,
        kind="ExternalOutput",
    )

    # chunk schedule per slice: (start_plane, n_planes). Slice 0 starts
    # with two 8-plane chunks so the compute pipeline fills early.
    sched0 = [(0, 8), (8, 8)] + [(16 * k, 16) for k in range(1, 8)]
    sched1 = [(16 * k, 16) for k in range(7)] + [(112, 8), (120, 8)]
    PREFETCH = 4

    with tile.TileContext(nc) as tc:
        with (
            tc.tile_pool(name="consts", bufs=1) as cpool,
            tc.tile_pool(name="inp", bufs=4) as ipool,
            tc.tile_pool(name="psum", bufs=2, space="PSUM") as ppool,
            tc.tile_pool(name="acc", bufs=3) as apool,
        ):
            # inputs ride the ACT HWDGE ring; outputs ride the SP ring, so
            # the two streams never FIFO-serialize behind each other.
            wh_t = cpool.tile([H, 128], F32, tag="wh")
            nc.scalar.dma_start(wh_t[:], wh.ap())

            jobs = [(0, c) for c in sched0] + [(1, c) for c in sched1]
            tiles = {}

            def issue_in(j):
                s, (p0, np_) = jobs[j]
                t = ipool.tile([H, 16 * W], F32, tag="chunk", name="chunk", bufs=5)
                nc.scalar.dma_start(
                    t[:, : np_ * W].rearrange("h (d w) -> h d w", w=W),
                    x.ap()[s][:, p0 : p0 + np_, :],
                )
                tiles[j] = t

            for j in range(min(PREFETCH, len(jobs))):
                issue_in(j)

            for j, (s, (p0, np_)) in enumerate(jobs):
                if j + PREFETCH < len(jobs):
                    issue_in(j + PREFETCH)
                chunk = tiles.pop(j)
                E = np_ // 2              # output e's in this chunk
                planes = chunk[:, : np_ * W].rearrange("h (d w) -> h d w", w=W)
                d_even = planes[:, 0::2, :]   # [128, E, 128]
                d_odd = planes[:, 1::2, :]

                # depth butterfly on raw input (SBUF->SBUF):
                # sums on DVE, diffs on GpSimd
                dsum = ipool.tile([H, 8 * W], F32, tag="dsum", name="dsum", bufs=3)
                ddiff = ipool.tile([H, 8 * W], F32, tag="ddiff", name="ddiff", bufs=3)
                nc.vector.tensor_add(
                    dsum[:, : E * W].rearrange("h (e w) -> h e w", w=W),
                    d_even, d_odd,
                )
                nc.gpsimd.tensor_sub(
                    ddiff[:, : E * W].rearrange("h (e w) -> h e w", w=W),
                    d_even, d_odd,
                )

                # H-stage matmuls (single weight set; fp32 N<=512)
                psum_lo = ppool.tile([128, 1024], F32, tag="lo", name="pl")
                psum_hi = ppool.tile([128, 1024], F32, tag="hi", name="ph")
                for half in range(E * W // 512):
                    sl = slice(half * 512, (half + 1) * 512)
                    nc.tensor.matmul(
                        psum_lo[:, sl], wh_t[:], dsum[:, sl],
                        start=True, stop=True,
                    )
                    nc.tensor.matmul(
                        psum_hi[:, sl], wh_t[:], ddiff[:, sl],
                        start=True, stop=True,
                    )

                # W-stage butterfly: TT may read only one PSUM operand, so
                # ScalarE stages the odd elements to SBUF; DVE reads the
                # even elements straight from PSUM.
                e0 = p0 // 2
                for src, t_sum, t_diff, nm in (
                    (psum_lo, 0, 1, "lo"),
                    (psum_hi, 2, 3, "hi"),
                ):
                    r = src[:, : E * W].rearrange(
                        "p (e w two) -> p e w two", two=2, w=W // 2
                    )
                    ev = r[:, :, :, 0]
                    od = r[:, :, :, 1]
                    sb_od = ipool.tile(
                        [128, 8 * (W // 2)], F32,
                        tag=f"sbod_{nm}", name=f"sb_od_{nm}", bufs=3,
                    )
                    od_sb = sb_od[:, : E * (W // 2)].rearrange(
                        "p (e w) -> p e w", w=W // 2
                    )
                    nc.scalar.copy(od_sb, od)
                    for t_out, is_sum in ((t_sum, True), (t_diff, False)):
                        acc = apool.tile(
                            [128, 8 * (W // 2)], F32,
                            tag=f"acc{t_out}", name=f"acc{t_out}", bufs=3,
                        )
                        out = acc[:, : E * (W // 2)].rearrange(
                            "p (e w) -> p e w", w=W // 2
                        )
                        if is_sum:
                            nc.vector.tensor_add(out, ev, od_sb)
                        else:
                            nc.vector.tensor_sub(out, ev, od_sb)
                        nc.sync.dma_start(
                            y.ap()[s, t_out][
                                :, e0 * (W // 2) : (e0 + E) * (W // 2)
                            ],
                            acc[:, : E * (W // 2)],
                        )

    return nc


_NC_CACHE = None


def _get_nc():
    global _NC_CACHE
    if _NC_CACHE is None:
        _NC_CACHE = build_bass()
    return _NC_CACHE


def _host_prep(inputs):
    l0 = np.asarray(inputs["matrix_low_0"], dtype=np.float64)   # (64,128)
    g0 = np.asarray(inputs["matrix_high_0"], dtype=np.float64)  # (64,128)
    l1 = np.asarray(inputs["matrix_low_1"], dtype=np.float64)   # (128,64)
    l2 = np.asarray(inputs["matrix_low_2"], dtype=np.float64)   # (64,128)
    v_w = l1[0, 0]
    v_d = l2[0, 0]
    ah = np.concatenate([l0, g0], axis=0)          # (128, 128) rows=bands
    wh = np.ascontiguousarray((ah.T * (v_w * v_d)).astype(np.float32))
    return wh


def run(inputs, trace=False, **kwargs):
    """Run the kernel; returns (bands_tuple, BassKernelResults)."""
    data = np.asarray(inputs["data"])
    assert data.shape == (2, 8, D, H, W) and data.dtype == np.float32
    wh = _host_prep(inputs)

    x = data.reshape(16, D, H, W)
    in_maps = []
    for k in range(N_CORES):
        # [s][d][h][w] -> [s][h][d][w] so device DMAs are dense
        xs = np.ascontiguousarray(x[2 * k : 2 * k + 2].transpose(0, 2, 1, 3))
        in_maps.append({"x": xs, "wh": wh})

    nc = _get_nc()
    res = run_bass_kernel_spmd(
        nc, in_maps, core_ids=list(range(N_CORES)), trace=trace, **kwargs
    )

    # Reassemble bands: y[k] is (2, 4, 128, 4096) ->
    # [slice][tile t=2*d_hi + w_hi][p' (h band halves)][e*64 + w'].
    bands = [np.empty((2, 8, D // 2, H // 2, W // 2), np.float32) for _ in range(8)]
    for k in range(N_CORES):
        yk = res.results[k]["y"].reshape(SLICES_PER_CORE, 4, 128, D // 2, W // 2)
        for s in range(SLICES_PER_CORE):
            ncf = 2 * k + s
            n, c = divmod(ncf, 8)
            for d_hi in (0, 1):
                for w_hi in (0, 1):
                    t = 2 * d_hi + w_hi
                    for h_hi in (0, 1):
                        band = 4 * d_hi + 2 * h_hi + w_hi
                        blk = yk[s, t, 64 * h_hi : 64 * h_hi + 64]  # [p', e, w']
                        bands[band][n, c] = blk.transpose(1, 0, 2)
    return tuple(bands), res


def kernel(**inputs):
    out, _ = run(inputs)
    return out
